# revision 32
# baseline (speedup 1.0000x reference)
"""Expert-parallel MoE feed-forward (top-2 routing) on 8 TRN2 NeuronCores.

Strategy: one expert per core (E == n_cores == 8). Token routing is part of
input sharding: host gathers each expert's assigned token activations
(transposed, bf16, pre-interleaved into SBUF tile layout) and feeds core e
only its tokens plus its expert's three weight matrices. Each core runs a
dense FFN
    out = (silu(x @ Wg^T) * (x @ Wu^T)) @ Wd^T
over its token batch in bf16 (fp32 PSUM accumulation), entirely from SBUF.
Host scatters per-core outputs back into the (T, A, D) result.

All DRAM inputs are host-pre-interleaved into SBUF tile layout so DMAs are
per-partition-contiguous ~100-250KB transfers (bigger single DMAs fan out
over too few DMA packet engines and crawl; each dma_start also costs
~0.6us of serial issue time on its queue engine). Each weight k-tile is
fed from two queues in parallel so the k-major arrival keeps ahead of the
PE during the ramp, and dummy matmuls on a zeroed tile warm the PE p-state
(1.2GHz -> 2.4GHz after ~3us busy) during the otherwise idle DMA ramp.
The tail is kept short: one output DMA per d-tile, the final d-tile split
so the last cast+DMA covers only ~128 columns, the final store issued from
the otherwise-idle scalar queue, and the exit sequence avoids blocking the
engines' expensive queue-drains from running ahead mid-kernel.
"""

import math
import sys
import types

import numpy as np
import ml_dtypes

T, D, H, E, A = 4096, 1024, 2048, 8, 2
N_CORES = 8
BF16 = ml_dtypes.bfloat16
KD = D // 128  # 8 k-tiles over the model dim
KH = H // 128  # 16 k-tiles over the hidden dim
# gate/up h-tile groups; each group g accumulates len(g) psum banks and its
# weight columns arrive as one DMA block.
GROUPS = [(0, 6), (6, 6), (12, 4)]  # (first h-tile, n h-tiles)

# Filled by kernel() with the BassKernelResults of the last device run so an
# external harness (test.py) can read exec_time_ns when tracing is on.
LAST_RESULT = None

_SHIMS_DONE = False


def _install_shims():
    """Environment fixes for running Bass/Tile SPMD kernels under axon."""
    global _SHIMS_DONE
    if _SHIMS_DONE:
        return
    _SHIMS_DONE = True

    # 1. NTFF profile hook (lets trace=True / BASS_TRACE=1 report exec_time_ns).
    if "antenv.axon_hooks" not in sys.modules:
        try:
            import antenv.axon_hooks  # noqa: F401  (real module present)
        except ImportError:
            _hook = None
            try:
                import trn_agent_boot.trn_boot as tb

                _hook = tb._ntff_profile_via_ctypes("/opt/axon/libaxon_pjrt.so")
            except Exception:
                _hook = None
            mod = types.ModuleType("antenv.axon_hooks")
            mod.get_axon_ntff_profile_hook = lambda: _hook
            sys.modules["antenv.axon_hooks"] = mod

    # 2. No artifact upload from a zero-egress container.
    from concourse import bass_utils

    bass_utils.upload_artifacts = lambda tmpdir: f"local:{tmpdir}"

    # 3. This walrus build allows only one sync-wait command on a CTRL
    # (Drain) instruction; split the tile-exit drain's waits onto nops.
    import concourse.tile as tile
    from concourse import mybir
    from concourse.vector_clock import ScopedClock

    if getattr(tile.TileContext._drain_and_barrier, "_is_patched", False):
        return

    def _patched_drain_and_barrier(self, tick_clock, wait_clock):
        nc = self.nc
        drain_inst = nc.sync.drain()
        wait_clock.add_sem_waits(
            drain_inst.ins, ScopedClock({None: tick_clock.global_clock})
        )
        ow = drain_inst.ins.sync_info.on_wait if drain_inst.ins.sync_info else None
        maxw = 1
        if ow and len(ow) > maxw:
            extra = list(ow[maxw:])
            del ow[maxw:]
            # All split waits stay on Sync: putting them on other engines
            # blocks those engines' barrier queue-DRAINs (expensive, up to
            # ~10us on gpsimd) from running ahead mid-kernel where they are
            # hidden under compute.
            for i in range(0, len(extra), maxw):
                nop = nc.sync.nop(hint="drain_split", nofuse=True)
                if nop.ins.sync_info is None:
                    nop.ins.sync_info = mybir.SyncInfo(on_wait=[], on_update=[])
                for w in extra[i : i + maxw]:
                    nop.ins.sync_info.on_wait.append(w)
        nc.all_engine_barrier()
        assert self.sems is not None
        popped = nc._tile_sem_poison_stack.pop()
        assert popped is self._sem_poison
        nc.clear_and_free_semaphores(list(self.sems.allocated().values()))

    _patched_drain_and_barrier._is_patched = True
    tile.TileContext._drain_and_barrier = _patched_drain_and_barrier


def _split_multi_waits(nc):
    """This walrus build allows one sync-wait command per instruction.

    Tile's sem assignment can attach several; move the extras onto nofuse
    NoOps inserted just before the instruction on the same engine (engines
    execute a block's instructions in order, so semantics are unchanged).
    """
    import bass_rust
    from concourse import mybir

    ctr = 0
    for f in nc.m.functions:
        for bb in f.blocks:
            new = []
            changed = False
            for inst in bb.instructions:
                si = inst.sync_info
                ow = si.on_wait if si else None
                if ow is not None and len(ow) > 1:
                    extra = list(ow[:-1])
                    del ow[:-1]
                    for w in extra:
                        ctr += 1
                        nop = bass_rust.InstNoOp()
                        nop.name = f"I-wsplit-{ctr}"
                        nop.engine = inst.engine
                        nop.sync_info = mybir.SyncInfo(on_wait=[w], on_update=[])
                        nop.bass_nofuse = True
                        new.append(nop)
                    changed = True
                new.append(inst)
            if changed:
                bb.instructions = new
    return nc


def _chunk_sizes(cap):
    """Split cap token columns into chunks of <=512 (PSUM bank limit)."""
    if cap <= 512:
        return [cap]
    first = 512
    rest = cap - first
    n = max(1, math.ceil(rest / 512))
    base = rest // n
    rem = rest - base * n
    return [first] + [base + (1 if i < rem else 0) for i in range(n)]


_NC_CACHE = {}


def _build_nc(cap):
    if cap in _NC_CACHE:
        return _NC_CACHE[cap]
    import concourse.bass as bass
    import concourse.tile as tile
    from concourse import mybir

    f32 = mybir.dt.float32
    bf16 = mybir.dt.bfloat16
    chunks = _chunk_sizes(cap)
    c_offs = []
    c0 = 0
    for cn in chunks:
        c_offs.append((c0, cn))
        c0 += cn

    GOFF = [0, 6144, 12288]  # dram col offset per gate/up weight group

    nc = bass.Bass()
    # x interleaved: per chunk block of 8*cn cols, s-major within the block.
    xT = nc.dram_tensor("xT", [128, KD * cap], bf16, kind="ExternalInput")
    # gate/up weights: per group block of 8*wcols, s-major within the block.
    wgT = nc.dram_tensor("wgT", [128, KD * H], bf16, kind="ExternalInput")
    wuT = nc.dram_tensor("wuT", [128, KD * H], bf16, kind="ExternalInput")
    # down weights: hk-major, 16 blocks of 1024 cols.
    wdT = nc.dram_tensor("wdT", [128, KH * D], bf16, kind="ExternalInput")
    out = nc.dram_tensor("out", [D, cap], bf16, kind="ExternalOutput")
    fin = 128 if chunks[-1] > 128 else 0  # final short slice of last d-tile

    with tile.TileContext(nc) as tc:
        with (
            tc.tile_pool(name="wpool", bufs=1) as wpool,
            tc.tile_pool(name="hpool", bufs=2) as hpool,
            tc.tile_pool(name="opool", bufs=4) as opool,
            tc.tile_pool(name="psum", bufs=2, space="PSUM") as psum,
        ):
            x_sb = [
                wpool.tile([128, KD, cn], bf16, tag=f"x{c}", name=f"x_sb{c}")
                for c, cn in enumerate(chunks)
            ]
            wg_sb = [
                wpool.tile([128, KD, gn * 128], bf16, tag=f"wg{g}", name=f"wg_sb{g}")
                for g, (_, gn) in enumerate(GROUPS)
            ]
            wu_sb = [
                wpool.tile([128, KD, gn * 128], bf16, tag=f"wu{g}", name=f"wu_sb{g}")
                for g, (_, gn) in enumerate(GROUPS)
            ]
            wd_sb = wpool.tile([128, KH, D], bf16, tag="wd", name="wd_sb")

            # PE p-state warmup: the PE clock ramps 1.2GHz -> 2.4GHz over
            # ~3us of continuous busy. Run dummy matmuls on a zeroed tile
            # during the otherwise-idle DMA ramp so the first real matmuls
            # start at full clock. gpsimd runs its preamble memsets before
            # any engine can issue DMAs, so the zero-fill lands earliest
            # there.
            dm = wpool.tile([128, 384], bf16, tag="dm", name="dm")
            nc.vector.memset(dm[:, :], 0)
            for _ in range(17):
                pw = psum.tile([128, 512], f32, tag="po", name="po_warm")
                nc.tensor.matmul(
                    pw[:, :256], dm[:, 0:128], dm[:, 128:384], start=True, stop=True
                )

            # --- input DMAs ---------------------------------------------
            # One dma_start per s-plane (~128-256KB, 2-D APs): big DMAs
            # fan out across too few DMA packet engines and crawl at
            # ~20-45GB/s, while many medium DMAs aggregate to ~358GB/s.
            # Issue order = consumption order; gate weights for chunk 0
            # split across two queues so the k-major arrival keeps ahead
            # of the PE during the ramp.
            def dma_w(eng, w_sb_g, w_dram, g, s, a, b):
                wcols = w_sb_g.shape[2]
                eng.dma_start(
                    w_sb_g[:, s, a:b],
                    w_dram[:, GOFF[g] + s * wcols + a : GOFF[g] + s * wcols + b],
                )

            def dma_x(eng, c, s):
                c0c, cnc = c_offs[c]
                eng.dma_start(
                    x_sb[c][:, s, :],
                    xT[:, KD * c0c + s * cnc : KD * c0c + (s + 1) * cnc],
                )

            # Each weight k-tile is fed from two queues in parallel (A/B
            # column halves) so the k-major arrival stays ahead of the PE,
            # which consumes 192KB every ~1.3us during the gate phase.
            # scalar gets a small leading slice plus the B-halves of group
            # 0 only, so it is free for the silu writers by ~13us.
            # sync's first issue lands earliest (~7.0us; scalar ~7.2,
            # gpsimd ~7.7 behind its preamble memsets), so the critical
            # first tiles go there: the j0 weight slice and half of x-s0.
            W0 = GROUPS[0][1] * 128  # 768
            c00, cn0 = c_offs[0]

            def dma_x0(eng, a, b):
                eng.dma_start(x_sb[0][:, 0, a:b], xT[:, a:b])

            dma_x0(nc.sync, 0, cn0 // 2)
            dma_x0(nc.scalar, cn0 // 2, cn0)
            dma_w(nc.gpsimd, wg_sb[0], wgT, 0, 0, 128, 384)
            dma_w(nc.sync, wg_sb[0], wgT, 0, 0, 0, 128)  # tiny: 1st matmul
            dma_w(nc.scalar, wg_sb[0], wgT, 0, 0, 384, W0)
            for s in range(1, KD):
                dma_w(nc.gpsimd, wg_sb[0], wgT, 0, s, 0, 384)
                dma_w(nc.scalar, wg_sb[0], wgT, 0, s, 384, W0)
                dma_x(nc.sync, 0, s)
            for s in range(KD):  # group 1: gpsimd A + sync B
                dma_w(nc.gpsimd, wg_sb[1], wgT, 1, s, 0, 384)
                dma_w(nc.sync, wg_sb[1], wgT, 1, s, 384, W0)
            for s in range(KD):  # group 2 (512 cols): gpsimd A + sync B
                dma_w(nc.gpsimd, wg_sb[2], wgT, 2, s, 0, 256)
                dma_w(nc.sync, wg_sb[2], wgT, 2, s, 256, 512)
            # up weights (needed once the gate phase of chunk 0 ends)
            for g in range(3):
                wcg = GROUPS[g][1] * 128
                for s in range(KD):
                    dma_w(nc.gpsimd, wu_sb[g], wuT, g, s, 0, wcg // 2)
                    dma_w(nc.sync, wu_sb[g], wuT, g, s, wcg // 2, wcg)
            for c in range(1, len(chunks)):
                for s in range(KD):
                    dma_x(nc.sync, c, s)
            # down weights (needed last), one DMA per hk
            for hk in range(KH):
                nc.gpsimd.dma_start(wd_sb[:, hk, :], wdT[:, hk * D : (hk + 1) * D])

            # --- compute ------------------------------------------------
            def gate_up(c, c0c, cn):
                # Phase 1: all gate matmuls; silu lands bf16 directly in h.
                # Phase 2: all up matmuls; h *= pu in place on the DVE.
                # Within a phase, k is the OUTER loop inside each group so
                # the weight consumption order matches DMA arrival during
                # the startup ramp.
                h_sb = hpool.tile([128, KH, cn], bf16, tag=f"h{c % 2}", name="h_sb")

                def phase(w_sb, writer):
                    for g, (g0, gn) in enumerate(GROUPS):
                        pp = [
                            psum.tile(
                                [128, 512], f32, tag=f"pp{j}", bufs=1, name=f"pp{j}"
                            )
                            for j in range(gn)
                        ]
                        for ki in range(KD):
                            for j in range(gn):
                                nc.tensor.matmul(
                                    pp[j][:, :cn],
                                    w_sb[g][:, ki, 128 * j : 128 * (j + 1)],
                                    x_sb[c][:, ki, :],
                                    start=(ki == 0),
                                    stop=(ki == KD - 1),
                                )
                        for j in range(gn):
                            writer(g0 + j, pp[j])

                def gate_writer(hi, pp):
                    nc.scalar.activation(
                        h_sb[:, hi, :],
                        pp[:, :cn],
                        mybir.ActivationFunctionType.Silu,
                    )

                def up_writer(hi, pp):
                    nc.vector.tensor_mul(h_sb[:, hi, :], h_sb[:, hi, :], pp[:, :cn])

                phase(wg_sb, gate_writer)
                phase(wu_sb, up_writer)
                return h_sb

            def down(h_sb, c0c, cn, last):
                for di in range(KD):
                    dsl = slice(128 * di, 128 * (di + 1))
                    # split the very last d-tile so the final store after
                    # the last matmul covers only ~128 columns
                    if last and di == KD - 1 and fin:
                        halves = [(0, cn - fin), (cn - fin, fin)]
                    else:
                        halves = [(0, cn)]
                    for hb, hn in halves:
                        po = psum.tile([128, 512], f32, tag="po", name="po")
                        for hk in range(KH):
                            nc.tensor.matmul(
                                po[:, :hn],
                                wd_sb[:, hk, dsl],
                                h_sb[:, hk, hb : hb + hn],
                                start=(hk == 0),
                                stop=(hk == KH - 1),
                            )
                        o = opool.tile([128, 512], bf16, tag="o", name="o")
                        nc.vector.tensor_copy(o[:, :hn], po[:, :hn])
                        # the very last store issues from scalar (idle by
                        # then) so sync can run its teardown semaphore-wait
                        # chain concurrently with this transfer
                        oeng = (
                            nc.scalar
                            if (last and di == KD - 1 and hb > 0)
                            else nc.sync
                        )
                        oeng.dma_start(
                            out[dsl, c0c + hb : c0c + hb + hn], o[:, :hn]
                        )

            # Software-pipelined emission: down(c) goes after gate_up(c+1) so
            # the PE can run chunk c+1's gate matmuls while the DVE finishes
            # chunk c's h tiles (h is double-buffered).
            prev = None
            for ci, (c0i, cni) in enumerate(c_offs):
                h_sb = gate_up(ci, c0i, cni)
                if prev is not None:
                    down(*prev, last=False)
                prev = (h_sb, c0i, cni)
            down(*prev, last=True)
    _split_multi_waits(nc)
    _NC_CACHE[cap] = nc
    return nc


def _interleave_w(wT):
    """[K, M] (K=k·128, k-tile-major rows) -> [128, K/128 * M] s-major."""
    K, M = wT.shape
    s = K // 128
    return np.ascontiguousarray(wT.reshape(s, 128, M).transpose(1, 0, 2)).reshape(
        128, s * M
    )


def kernel(x, expert_indices, w_gate, w_up, w_down):
    global LAST_RESULT
    _install_shims()
    from concourse import bass_utils

    x = np.asarray(x)
    ei = np.asarray(expert_indices).astype(np.int64)
    w_gate = np.asarray(w_gate)
    w_up = np.asarray(w_up)
    w_down = np.asarray(w_down)

    flat = ei.reshape(-1)  # pair p = t*A + a  ->  expert id
    # Dedup: a (token, slot) pair whose expert already appears in an earlier
    # slot of the same token produces an identical output row — compute the
    # first occurrence only and copy the result to the duplicates afterward.
    keep = np.ones(T * A, dtype=bool)
    for a in range(1, A):
        dup_any = np.zeros(T, dtype=bool)
        for b in range(a):
            dup_any |= ei[:, a] == ei[:, b]
        keep[a::A] = ~dup_any[:T]
    kept = np.nonzero(keep)[0]
    flat_kept = flat[kept]
    counts = np.bincount(flat_kept, minlength=E)
    order = np.argsort(flat_kept, kind="stable")
    starts = np.zeros(E + 1, dtype=np.int64)
    np.cumsum(counts, out=starts[1:])
    cap = int(counts.max())
    cap = max(cap, 128)
    chunks = _chunk_sizes(cap)

    idx_per_core = []
    in_maps = []
    for e in range(E):
        idx = kept[order[starts[e] : starts[e + 1]]]  # original pair ids
        idx_per_core.append(idx)
        tok = idx // A
        xeT = np.zeros((D, cap), dtype=BF16)
        xeT[:, : len(idx)] = x[tok].T.astype(BF16)
        # interleave x per chunk: block of [s, cn] per partition
        xr = xeT.reshape(KD, 128, cap)
        xb = []
        c0 = 0
        for cn in chunks:
            xb.append(xr[:, :, c0 : c0 + cn].transpose(1, 0, 2).reshape(128, KD * cn))
            c0 += cn
        x_il = np.ascontiguousarray(np.concatenate(xb, axis=1))

        wgT = np.ascontiguousarray(w_gate[e].T).astype(BF16)  # [D, H]
        wuT = np.ascontiguousarray(w_up[e].T).astype(BF16)
        wdT = np.ascontiguousarray(w_down[e].T).astype(BF16)  # [H, D]
        # gate/up: group blocks [768, 768, 512] cols, each s-major
        wr_g = wgT.reshape(KD, 128, H)
        wr_u = wuT.reshape(KD, 128, H)
        gb_g, gb_u = [], []
        for g0, gn in GROUPS:
            cs = slice(128 * g0, 128 * (g0 + gn))
            gb_g.append(wr_g[:, :, cs].transpose(1, 0, 2).reshape(128, KD * gn * 128))
            gb_u.append(wr_u[:, :, cs].transpose(1, 0, 2).reshape(128, KD * gn * 128))
        wg_il = np.ascontiguousarray(np.concatenate(gb_g, axis=1))
        wu_il = np.ascontiguousarray(np.concatenate(gb_u, axis=1))
        wd_il = _interleave_w(wdT)

        in_maps.append({"xT": x_il, "wgT": wg_il, "wuT": wu_il, "wdT": wd_il})

    nc = _build_nc(cap)
    res = bass_utils.run_bass_kernel_spmd(nc, in_maps, core_ids=list(range(N_CORES)))
    LAST_RESULT = res

    out = np.zeros((T * A, D), dtype=np.float32)
    for e in range(E):
        idx = idx_per_core[e]
        oT = np.asarray(res.results[e]["out"])  # [D, cap] bf16
        out[idx] = oT[:, : len(idx)].T.astype(np.float32)
    out = out.reshape(T, A, D)
    for a in range(1, A):  # fill duplicate slots from their first occurrence
        for b in range(a):
            m = ei[:, a] == ei[:, b]
            if b > 0:
                for c in range(b):
                    m &= ei[:, b] != ei[:, c]  # b is itself the first occurrence
            out[m, a] = out[m, b]
    return out
